# revision 13
# baseline (speedup 1.0000x reference)
"""ChebConv (order-4) GNN layer on 8 Trainium2 NeuronCores.

Reference computation (fp32):
    T0 = x, T1 = G x, Tk = 2 G T{k-1} - T{k-2}
    out = sum_k Tk @ W[k]          # [N, F] with N=10000, F=32
Rewritten in the power basis: y0 = x, yk = G y{k-1},
    out = sum_k yk @ Wp[k]  with
    Wp = [W0 - W2, W1 - 3 W3, 2 W2, 4 W3]   (exact modulo fp reassociation)

Strategy (v5):
  * G, the per-hop node features, and Wp[1:] are plain bf16 with fp32
    PSUM accumulation (rel-err ~4e-3 vs the 2e-2 gate); this halves HBM
    bytes and triples PE throughput vs the original hi/lo-split kernel.
  * Row-shard G over 8 cores (1280 padded cols of G^T each, pad
    10000 -> 10240). Per core, 56 of the 80 128-row j-chunks of the
    G^T slice (~18 MB bf16) are pinned in SBUF during hop 1 and reused
    by hops 2-3, which then stream only ~6.5 MB each: hop 1 runs at
    the HBM roofline (~75 us), hops 2-3 at the PE roofline (~45 us).
  * HWDGE trigger instructions cost ~0.6 us on the issuing engine and
    block on descriptor backpressure, so the host lays G out as
    partition-major per-sweep images: every pinned-set load is ONE
    plain 2D DMA with multi-KB per-partition descriptors (9 triggers
    for all of hop 1's pins), and the 24 streamed chunks load as one
    2D batch per (sweep, part) into a 4-deep ring.
  * Sweeps run in gather-part order [2-chunk part, 4, 4] (host permutes
    G^T/x^T columns so sweep columns stay contiguous). Hops 2-3 fire a
    partial AllGather per sweep, and each hop consumes j-chunks in the
    same part order, so hop k+1 starts as soon as hop k's first
    (smallest, earliest) gather lands. Hop 1's gathers are instead
    DEFERRED and merged into a single AllGather of all of y1 at hop
    end: collectives execute serially on the CC cores behind a
    ~40-85 us one-time init barrier (starts ~21 us into the NEFF), and
    any collective executing while hop 1 still streams G starves the
    HWDGE drain and convoys the whole hop. A tiny dummy AllGather
    issued first absorbs the first-call warmup during hop 1's tail.
    Reloads of gathered y into per-part v tiles ride the scalar queue
    so the CC queue runs gathers back-to-back.
  * Each hop computes y_k^T in 3 sweeps (one PSUM bank per sweep): per
    j-chunk one bf16 matmul (lhsT = v[j] [128,32], rhs = G^T tile
    [128,<=512]) accumulates over all 80 chunks; the epilogue copies
    PSUM to bf16 y16, adds the bf16 Wp_k term into the fp32 output
    accumulator (bf16 rhs streams at full PE rate; fp32 runs LOW_HIGH
    at half rate), PE-transposes the sweep rows and stages the gather
    input. The k=0 term uses the host fp32 xT slice.
  * Output is returned transposed and column-permuted ([32, 1280] per
    core); the host concatenates, un-permutes, transposes and drops
    padding.
"""

import sys

if "/opt/trn_rl_repo" not in sys.path:
    sys.path.insert(0, "/opt/trn_rl_repo")

import numpy as np

N = 10000
F = 32
ORDER = 4
NCORES = 8
P = 128
NP = 10240  # padded node count: divisible by NCORES * P
RPC = NP // NCORES  # rows per core (1280)
JC = NP // P  # global 128-row chunks (80)
MC = RPC // P  # local 128-row chunks per core (10)

# gather parts in sweep order; per part: natural m-chunks (host permutes
# columns to this order), pinned m-chunks, streamed m-chunk
PART_MS = [[8, 9], [0, 1, 2, 3], [4, 5, 6, 7]]
PIN_MS = [[8], [0, 1, 2], [4, 5, 6]]
STR_MS = [9, 3, 7]
NEW_MS = [m for ms in PART_MS for m in ms]  # host column permutation

_CACHE = {}


def _build(np_total, ncores):
    from concourse import bacc, masks, mybir, tile

    rpc = np_total // ncores
    jc = np_total // P
    mc = rpc // P
    f32 = mybir.dt.float32
    bf16 = mybir.dt.bfloat16
    nfc = len(PART_MS)
    parts = []
    s = 0
    for ms in PART_MS:
        parts.append((s // P, len(ms)))
        s += len(ms) * P
    fchunks = [(m0 * P, nm * P) for (m0, nm) in parts]
    vcols = [ncores * nm * F for (m0, nm) in parts]
    # stage/v column offset of each part (units of F cols)
    part_off = [0, 0, 0]
    for pi in range(1, nfc):
        part_off[pi] = part_off[pi - 1] + len(PART_MS[pi - 1])

    nc = bacc.Bacc(
        "TRN2", target_bir_lowering=False, debug=False, num_devices=ncores
    )
    # pinned G, one partition-major image per sweep: row p holds, for
    # each part pi then each (c, m-in-run) a, that chunk's sweep-i
    # column slice: [P, 56 * l_i]
    g_pins = [
        nc.dram_tensor(
            f"g_pin{i}", [P, ncores * 7 * l], bf16, kind="ExternalInput"
        ).ap()
        for i, (s, l) in enumerate(fchunks)
    ]
    # streamed G, same layout: [P, 24 * l_i], parts-major
    g_strs = [
        nc.dram_tensor(
            f"g_str{i}", [P, ncores * 3 * l], bf16, kind="ExternalInput"
        ).ap()
        for i, (s, l) in enumerate(fchunks)
    ]
    # column offset (elements) of part pi inside g_pins[i] / g_strs[i]
    pin_coff = [0, 0, 0]
    str_coff = [0, 0, 0]
    for pi in range(1, nfc):
        pin_coff[pi] = pin_coff[pi - 1] + ncores * len(PIN_MS[pi - 1])
        str_coff[pi] = str_coff[pi - 1] + ncores

    xv = nc.dram_tensor("xv", [P, sum(vcols)], bf16, kind="ExternalInput").ap()
    xt = nc.dram_tensor("xt", [F, rpc], f32, kind="ExternalInput").ap()
    wp = nc.dram_tensor("wp", [F, ORDER * F], f32, kind="ExternalInput").ap()
    out_t = nc.dram_tensor("outT", [F, rpc], f32, kind="ExternalOutput").ap()

    m2part = {}
    for pi, ms in enumerate(PART_MS):
        for ml, m in enumerate(ms):
            m2part[m] = (pi, ml)

    # consumption order: parts in gather-firing order; within a part
    # pinned chunks (c-major), then the streamed batch
    jorder = []
    for pi in range(nfc):
        jorder += [c * mc + m for c in range(ncores) for m in PIN_MS[pi]]
        jorder += [c * mc + STR_MS[pi] for c in range(ncores)]

    with tile.TileContext(nc) as tc:
        with (
            tc.tile_pool(name="const", bufs=1) as constp,
            tc.tile_pool(name="gsp", bufs=4) as gsp,
            tc.tile_pool(name="vp", bufs=2) as vp,
            tc.tile_pool(name="sb", bufs=2) as sb,
            tc.tile_pool(name="ps_hop", bufs=1, space="PSUM") as ps_hop,
            tc.tile_pool(name="ps_tp", bufs=2, space="PSUM") as ps_tp,
            tc.tile_pool(name="ps_w", bufs=2, space="PSUM") as ps_w,
            tc.tile_pool(name="dram", bufs=2, space="DRAM") as dram,
        ):
            ident = constp.tile([P, P], f32)
            masks.make_identity(nc, ident[:])
            xt_sb = constp.tile([F, rpc], f32)
            nc.scalar.dma_start(xt_sb[:], xt)
            w_sb = constp.tile([F, ORDER * F], f32)
            nc.scalar.dma_start(w_sb[:], wp)
            out_sb = constp.tile([F, rpc], f32)
            ident16 = constp.tile([F, F], bf16)
            nc.vector.tensor_copy(ident16[:], ident[0:F, 0:F])
            w16 = constp.tile([F, ORDER * F], bf16)
            nc.vector.tensor_copy(w16[:], w_sb[:])

            # pinned G: one tile per (part, sweep), one 2D DMA each
            pin = {}
            for pi in range(nfc):
                na = ncores * len(PIN_MS[pi])
                for i, (s, l) in enumerate(fchunks):
                    pin[(pi, i)] = constp.tile(
                        [P, na * l], bf16, name=f"pin{pi}_{i}"
                    )

            # v holds y_{k-1} as bf16, one tile per part so next-hop
            # matmuls only depend on the gather that produced them
            v_parts = []
            off = 0
            for i, w_ in enumerate(vcols):
                vt = vp.tile([P, w_], bf16, tag=f"v{i}", name=f"v{i}")
                nc.scalar.dma_start(vt[:], xv[:, off : off + w_])
                off += w_
                v_parts.append(vt)

            def v_of(vps, j):
                c, m = j // mc, j % mc
                pi, ml = m2part[m]
                nm = len(PART_MS[pi])
                col = (c * nm + ml) * F
                return vps[pi][:, col : col + F]

            # k = 0 contribution: out^T = Wp_0^T @ x^T (pure fp32)
            for s, l in fchunks:
                pw = ps_w.tile([F, l], f32, tag="pw")
                nc.tensor.matmul(
                    pw[:], lhsT=w_sb[:, 0:F], rhs=xt_sb[:, s : s + l],
                    start=True, stop=True,
                )
                nc.vector.tensor_copy(out_sb[:, s : s + l], pw[:])

            def all_gather(cc_in_src, nmtot, tag):
                cc_in = dram.tile(
                    [P, nmtot * F], bf16, tag=f"ci{tag}", name=f"ci{tag}"
                )
                cc_out = dram.tile(
                    [ncores * P, nmtot * F], bf16, tag=f"co{tag}",
                    name=f"co{tag}", addr_space="Shared",
                )
                nc.scalar.dma_start(cc_in[:], cc_in_src)
                nc.gpsimd.collective_compute(
                    "AllGather",
                    mybir.AluOpType.bypass,
                    replica_groups=[list(range(ncores))],
                    ins=[cc_in.opt()],
                    outs=[cc_out.opt()],
                )
                return cc_out

            def reload(cc_out, col0, nm, v_dst):
                # v part reload rides SWDGE (gpsimd): software DGE has
                # its own completion-semaphore space, so this gather-
                # gated DMA cannot poison HWDGE completion-ordering
                # semaphores shared with the G stream
                nc.gpsimd.dma_start(
                    v_dst[:].rearrange("p (c m) -> p c m", c=ncores),
                    cc_out[:, col0 * F : (col0 + nm) * F].rearrange(
                        "(c p) m -> p c m", p=P
                    ),
                )

            for k in range(1, ORDER):
                v_cur = v_parts
                if k < ORDER - 1:
                    v_next = [
                        vp.tile([P, w_], bf16, tag=f"v{i}", name=f"vn{i}")
                        for i, w_ in enumerate(vcols)
                    ]
                y16 = sb.tile([F, rpc], bf16, tag="y16")
                if k == ORDER - 1:
                    # last hop: no gathers downstream, so consume part-
                    # major across all 3 sweeps (3 open PSUM banks);
                    # only the final part's chunks remain after the
                    # last reload lands, instead of 2 whole sweeps
                    # queued behind the first sweep's stalled tail
                    hps = {}
                    sbt3 = {}
                    for i, (s, l) in enumerate(fchunks):
                        hps[i] = ps_hop.tile(
                            [F, l], f32, tag=f"hop{i}", name=f"hp{i}"
                        )
                    for pi in range(nfc):
                        for i, (s, l) in enumerate(fchunks):
                            t = gsp.tile(
                                [P, ncores * 512], bf16, tag="gs", name="gs"
                            )
                            nc.sync.dma_start(
                                t[:, 0 : ncores * l],
                                g_strs[i][
                                    :,
                                    str_coff[pi] * l
                                    : (str_coff[pi] + ncores) * l,
                                ],
                            )
                            sbt3[(pi, i)] = t
                    jn3 = {i: 0 for i in range(nfc)}
                    for pi in range(nfc):
                        pjs = [
                            c * mc + m
                            for c in range(ncores)
                            for m in PIN_MS[pi]
                        ]
                        pjs += [c * mc + STR_MS[pi] for c in range(ncores)]
                        for i, (s, l) in enumerate(fchunks):
                            for j in pjs:
                                c, m = j // mc, j % mc
                                if m in STR_MS:
                                    g = sbt3[(pi, i)][:, c * l : (c + 1) * l]
                                else:
                                    a = (
                                        c * len(PIN_MS[pi])
                                        + PIN_MS[pi].index(m)
                                    )
                                    g = pin[(pi, i)][:, a * l : (a + 1) * l]
                                nc.tensor.matmul(
                                    hps[i][:], lhsT=v_of(v_cur, j), rhs=g,
                                    start=(jn3[i] == 0),
                                    stop=(jn3[i] == jc - 1),
                                )
                                jn3[i] += 1
                    for i, (s, l) in enumerate(fchunks):
                        nc.vector.tensor_copy(y16[:, s : s + l], hps[i][:])
                        pw = ps_w.tile([F, l], f32, tag="pw")
                        nc.tensor.matmul(
                            pw[:], lhsT=w16[:, k * F : (k + 1) * F],
                            rhs=y16[:, s : s + l], start=True, stop=True,
                        )
                        nc.vector.tensor_add(
                            out_sb[:, s : s + l], out_sb[:, s : s + l], pw[:]
                        )
                    continue
                for i, (s, l) in enumerate(fchunks):
                    # loads in consumption order per part: hop-1 pin
                    # image chunk, then the streamed batch (all 2D)
                    sbt = {}
                    for pi in range(nfc):
                        if k == 1:
                            na = ncores * len(PIN_MS[pi])
                            for a0 in range(0, na, ncores):
                                nc.sync.dma_start(
                                    pin[(pi, i)][:, a0 * l : (a0 + ncores) * l],
                                    g_pins[i][
                                        :,
                                        (pin_coff[pi] + a0) * l
                                        : (pin_coff[pi] + a0 + ncores) * l,
                                    ],
                                )
                        t = gsp.tile(
                            [P, ncores * 512], bf16, tag="gs", name="gs"
                        )
                        nc.sync.dma_start(
                            t[:, 0 : ncores * l],
                            g_strs[i][:, str_coff[pi] * l : (str_coff[pi] + ncores) * l],
                        )
                        sbt[pi] = t
                    hp = ps_hop.tile([F, l], f32, tag=f"hop{i}", name=f"hp{i}")
                    for jn, j in enumerate(jorder):
                        c, m = j // mc, j % mc
                        pi, ml = m2part[m]
                        if m in STR_MS:
                            g = sbt[pi][:, c * l : (c + 1) * l]
                        else:
                            a = c * len(PIN_MS[pi]) + PIN_MS[pi].index(m)
                            g = pin[(pi, i)][:, a * l : (a + 1) * l]
                        nc.tensor.matmul(
                            hp[:], lhsT=v_of(v_cur, j), rhs=g,
                            start=(jn == 0), stop=(jn == jc - 1),
                        )
                    # sweep epilogue: PSUM -> bf16 y16, Wp contribution
                    nc.vector.tensor_copy(y16[:, s : s + l], hp[:])
                    pw = ps_w.tile([F, l], f32, tag="pw")
                    nc.tensor.matmul(
                        pw[:], lhsT=w16[:, k * F : (k + 1) * F],
                        rhs=y16[:, s : s + l], start=True, stop=True,
                    )
                    nc.vector.tensor_add(
                        out_sb[:, s : s + l], out_sb[:, s : s + l], pw[:]
                    )
                    if k < ORDER - 1:
                        # transpose sweep rows to natural layout; parts
                        # 1+2 stage into one buffer and share a single
                        # merged gather (4 collectives total instead of
                        # 6 - each costs a ~13 us ncfw/barrier floor)
                        m0, nm = parts[i]
                        if i == 0:
                            st = sb.tile(
                                [P, nm * F], bf16, tag="stage0",
                                name="stage0",
                            )
                            stage = st[:]
                            soff = 0
                        else:
                            if i == 1:
                                st12 = sb.tile(
                                    [P, 8 * F], bf16, tag="stage12",
                                    name="stage12",
                                )
                            stage = st12[:]
                            soff = parts[i][0] - parts[1][0]
                        for mm in range(nm):
                            m = m0 + mm
                            tp = ps_tp.tile([P, F], bf16, tag="tp", name="tp")
                            nc.tensor.transpose(
                                tp[:], y16[:, m * P : (m + 1) * P],
                                ident16[:],
                            )
                            nc.vector.tensor_copy(
                                stage[:, (soff + mm) * F : (soff + mm + 1) * F],
                                tp[:],
                            )
                        if i == 0:
                            cc_out = all_gather(stage, nm, "p0")
                            reload(cc_out, 0, nm, v_next[0])
                        elif i == 2:
                            cc_out = all_gather(stage, 8, "p12")
                            reload(cc_out, 0, len(PART_MS[1]), v_next[1])
                            reload(
                                cc_out, len(PART_MS[1]), len(PART_MS[2]),
                                v_next[2],
                            )
                if k < ORDER - 1:
                    v_parts = v_next

            nc.scalar.dma_start(out_t, out_sb[:])

    nc.compile()
    return nc


def get_nc(np_total=NP, ncores=NCORES):
    key = (np_total, ncores)
    if key not in _CACHE:
        _CACHE[key] = _build(np_total, ncores)
    return _CACHE[key]


def prep_inputs(x, gso, weight, np_total=NP, ncores=NCORES):
    """Host-side shard prep. Returns in_maps for run_bass_kernel_spmd."""
    import ml_dtypes

    bf = ml_dtypes.bfloat16
    n = x.shape[0]
    rpc = np_total // ncores
    mc = rpc // P

    x = np.asarray(x, dtype=np.float32)
    gso = np.asarray(gso, dtype=np.float32)
    weight = np.asarray(weight, dtype=np.float32)

    wp = np.concatenate(
        [
            weight[0] - weight[2],
            weight[1] - 3.0 * weight[3],
            2.0 * weight[2],
            4.0 * weight[3],
        ],
        axis=1,
    ).astype(np.float32)  # [F, ORDER*F]

    xpad = np.zeros((np_total, F), dtype=np.float32)
    xpad[:n] = x
    gpad = np.zeros((np_total, np_total), dtype=np.float32)
    gpad[:n, :n] = gso
    g16 = gpad.astype(bf)
    x16 = xpad.astype(bf)

    def part_x(ms):
        return (
            x16.reshape(ncores, mc, P, F)[:, ms]
            .transpose(2, 0, 1, 3)
            .reshape(P, ncores * len(ms) * F)
        )

    xv = np.ascontiguousarray(np.concatenate([part_x(ms) for ms in PART_MS], 1))

    fchunks = []
    s = 0
    for ms in PART_MS:
        fchunks.append((s, len(ms) * P))
        s += len(ms) * P

    in_maps = []
    for c in range(ncores):
        rows = slice(c * rpc, (c + 1) * rpc)
        gt = np.ascontiguousarray(g16[rows, :].T)  # [np_total, rpc]
        # permute output columns to sweep order
        gt = gt.reshape(np_total, mc, P)[:, NEW_MS].reshape(np_total, rpc)
        gt4 = gt.reshape(ncores, mc, P, rpc)
        # partition-major per-sweep images: [P, chunks * l]
        pin_rows = np.stack(
            [gt4[cb, m] for ms in PIN_MS for cb in range(ncores) for m in ms]
        )  # [56, P, rpc]
        str_rows = np.stack(
            [gt4[cb, m] for m in STR_MS for cb in range(ncores)]
        )  # [24, P, rpc]
        m = {"xv": xv, "wp": wp}
        for i, (s, l) in enumerate(fchunks):
            m[f"g_pin{i}"] = np.ascontiguousarray(
                pin_rows[:, :, s : s + l].transpose(1, 0, 2).reshape(P, -1)
            )
            m[f"g_str{i}"] = np.ascontiguousarray(
                str_rows[:, :, s : s + l].transpose(1, 0, 2).reshape(P, -1)
            )
        xtc = np.ascontiguousarray(xpad[rows, :].T)  # [F, rpc] fp32
        m["xt"] = np.ascontiguousarray(
            xtc.reshape(F, mc, P)[:, NEW_MS].reshape(F, rpc)
        )
        in_maps.append(m)
    return in_maps


def assemble_output(results, n=N, ncores=NCORES):
    inv = np.argsort(NEW_MS)
    outs = []
    for c in range(ncores):
        o = results[c]["outT"]  # [F, RPC] permuted cols
        outs.append(o.reshape(F, MC, P)[:, inv].reshape(F, RPC))
    out_t = np.concatenate(outs, axis=1)
    return np.ascontiguousarray(out_t.T[:n]).astype(np.float32)


def kernel(x, gso, weight):
    import time

    from concourse import bass_utils

    nc = get_nc()
    in_maps = prep_inputs(x, gso, weight)
    last_err = None
    for attempt in range(3):
        try:
            res = bass_utils.run_bass_kernel_spmd(
                nc, in_maps, core_ids=list(range(NCORES))
            )
            return assemble_output(res.results)
        except Exception as e:  # transient device wedge: retry
            last_err = e
            time.sleep(5.0 * (attempt + 1))
    raise last_err
